# revision 63
# baseline (speedup 1.0000x reference)
"""Trainium2 Bass kernel for GQA causal self-attention (nn_CausalSelfAttention).

Model (hardcoded from the problem spec):
  B=2, T=2048, C=2048, n_head=32, n_kv=8, hs=64
  qkv = x @ w_attn.T + b_attn ; causal GQA attention ; y @ w_proj.T + b_proj

Sharding over 8 cores: core g handles batch b = g//4 and head-group grp = g%4
(8 q-heads, 2 kv-heads per core).  c_attn columns and c_proj rows are split
head-wise; the c_proj partial sums are reduced on the host (the "all-reduce").

Device layout notes:
 - All matmuls contract over the partition dim.  Host pre-transposes x and the
   weight slices so no on-device transposes are needed.
 - Scores are computed K-stationary: S.T tile [tk, tq] = kT.T @ q, so softmax's
   P.T is directly the moving operand of the PV matmul (no P transpose).
 - exp without max-subtraction (scores are ~N(0,1); exp is safe in f32).
 - softmax denominator = ones-row appended to V (row 64 of the PV output);
   normalization multiplies by a [1,tq] reciprocal broadcast to 64 partitions
   via gpsimd partition_broadcast.
 - q rows are stored interleaved ([h0,h4 | h1,h5 | h2,h6 | h3,h7] 64-row
   blocks) so each head's q/k share the same SBUF base partition (0 or 64).
 - heads are processed in pairs (h, h+4): their score matmuls use PE array
   rows 0:63 vs 64:127 (tile_position row groups) and are emitted adjacently
   so the hardware overlaps them; both land in one 2-bank psum tile so a
   single exp covers the pair.
 - block-causal: only tk-tiles <= the tq-tile are computed; in diagonal
   blocks the fully-masked leading columns are skipped in the matmul, exp,
   and PV (psum/pt slots are pre-zeroed so skipped regions stay finite).
 - emission is software-pipelined: projections for token-slice j+1 and
   c_proj for slice j-1 are round-robined between the attention units of
   slice j, keeping the PE busy while exps drain.
"""

import sys
import numpy as np
import ml_dtypes
from contextlib import ExitStack

for _p in ("/opt/trn_rl_repo", "/root/.axon_site/_ro/trn_rl_repo"):
    if _p not in sys.path:
        sys.path.append(_p)

import concourse.mybir as mybir
import concourse.tile as tile
from concourse import bacc
from concourse.bass_utils import run_bass_kernel_spmd

BF16 = mybir.dt.bfloat16
F32 = mybir.dt.float32
NPBF16 = ml_dtypes.bfloat16

B, T, C = 2, 2048, 2048
N_HEAD, N_KV, HS = 32, 8, 64
NE = 2048
N_CORES = 8
HL = 8          # q heads per core
KVL = 2         # kv heads per core
P = 128
TQ = 512        # tq tile (matmul moving width)
NJ = T // TQ    # 4 tq tiles
NT = T // P     # 16 token tiles
KC = C // P     # 16 contraction tiles over channels
QROWS = HL * HS          # 512 local q rows
KROWS = KVL * HS         # 128 local k rows
WCOLS = QROWS + 2 * KROWS  # 768 local w_attn rows

# position-block -> local head: q_sb m-tile mt rows [0:64]=head mt, [64:128]=head mt+4
Q_ORDER = [0, 4, 1, 5, 2, 6, 3, 7]

_CACHE = {}


def _build_program():
    nc = bacc.Bacc("TRN2", target_bir_lowering=False, debug=False)

    xT_d = nc.dram_tensor("xT", [C, T], BF16, kind="ExternalInput")
    wqkvT_d = nc.dram_tensor("wqkvT", [C, WCOLS], BF16, kind="ExternalInput")
    wpT_d = nc.dram_tensor("wpT", [QROWS, C], BF16, kind="ExternalInput")
    bq_d = nc.dram_tensor("bq", [4, P], F32, kind="ExternalInput")
    bk_d = nc.dram_tensor("bk", [1, P], F32, kind="ExternalInput")
    out_d = nc.dram_tensor("out", [T, C], F32, kind="ExternalOutput")

    with tile.TileContext(nc) as tc:
        with ExitStack() as ctx:
            _emit(ctx, tc, nc, xT_d, wqkvT_d, wpT_d, bq_d, bk_d, out_d)
    nc.compile()
    return nc


def _emit(ctx, tc, nc, xT_d, wqkvT_d, wpT_d, bq_d, bk_d, out_d):
    ExpF = mybir.ActivationFunctionType.Exp
    add = mybir.AluOpType.add
    mult = mybir.AluOpType.mult

    persist = ctx.enter_context(tc.tile_pool(name="persist", bufs=1))
    ppa = ctx.enter_context(tc.tile_pool(name="ppa", bufs=2, space="PSUM"))
    pps = ctx.enter_context(tc.tile_pool(name="pps", bufs=2, space="PSUM"))
    ppo = ctx.enter_context(tc.tile_pool(name="ppo", bufs=2, space="PSUM"))
    ptpool = ctx.enter_context(tc.tile_pool(name="pt", bufs=6))
    rcpool = ctx.enter_context(tc.tile_pool(name="rc", bufs=4))
    bcpool = ctx.enter_context(tc.tile_pool(name="bc", bufs=4))
    mkpool = ctx.enter_context(tc.tile_pool(name="mk", bufs=2))
    outpool = ctx.enter_context(tc.tile_pool(name="os", bufs=6))
    obpool = ctx.enter_context(tc.tile_pool(name="ob", bufs=5))

    # ---- persistent SBUF tensors ----
    xT_sb = persist.tile([P, KC * T], BF16, tag="xT")
    wqkv_sb = persist.tile([P, KC * WCOLS], BF16, tag="wqkv")
    wp_sb = persist.tile([P, 4 * C], BF16, tag="wp")
    q_sb = persist.tile([P, 4 * T], BF16, tag="q")
    kT_sb = persist.tile([P, T], BF16, tag="k")
    v_sb = persist.tile([P, NT * 130], BF16, tag="v")
    y_sb = persist.tile([P, 4 * T], BF16, tag="y")
    bq_sb = persist.tile([P, 4], F32, tag="bq")
    bk_sb = persist.tile([P, 1], F32, tag="bk")
    # mask variants for diagonal blocks, doubled for the head-pair layout:
    # maskv[r][x, y] = 1 if (y mod 512)-x-128r >= 0 else 0
    maskv = [persist.tile([P, 2 * TQ], BF16, tag=f"mask{r}", name=f"mask{r}")
             for r in range(4)]

    # ---- input DMAs ----
    nc.sync.dma_start(bq_sb[:], bq_d.ap().rearrange("t p -> p t"))
    nc.sync.dma_start(bk_sb[:], bk_d.ap().rearrange("t p -> p t"))
    # critical first window: weight tile k and its first xT chunk alternate
    for k in range(KC):
        nc.sync.dma_start(wqkv_sb[:, k * WCOLS:(k + 1) * WCOLS],
                          wqkvT_d.ap()[k * P:(k + 1) * P, :])
        nc.sync.dma_start(xT_sb[:, k * T: k * T + TQ],
                          xT_d.ap()[k * P:(k + 1) * P, 0:TQ])
    # remaining xT token chunks (proj(n) starts after chunk n)
    for n in range(1, NJ):
        for k in range(KC):
            nc.sync.dma_start(xT_sb[:, k * T + n * TQ: k * T + (n + 1) * TQ],
                              xT_d.ap()[k * P:(k + 1) * P, n * TQ:(n + 1) * TQ])
    for k in range(4):
        nc.sync.dma_start(wp_sb[:, k * C:(k + 1) * C],
                          wpT_d.ap()[k * P:(k + 1) * P, :])

    # ---- constants ----
    for r in range(4):
        mf = mkpool.tile([P, TQ], F32, tag="mf")
        nc.gpsimd.memset(mf[:], 1.0)
        nc.gpsimd.affine_select(
            out=mf[:], in_=mf[:], compare_op=mybir.AluOpType.is_ge,
            fill=0.0, base=-128 * r, pattern=[[1, TQ]], channel_multiplier=-1)
        nc.scalar.copy(maskv[r][:, 0:TQ], mf[:])
        nc.scalar.copy(maskv[r][:, TQ:2 * TQ], mf[:])
    nc.vector.memset(v_sb[:], 1.0)  # ones columns; data cols overwritten below
    # pre-zero the score psum slots: diagonal blocks are computed at reduced
    # width, so the masked-off region must hold finite values for exp()
    for w in range(2):
        pwarm = pps.tile([P, 2 * TQ], F32, tag="ps", name="pswarm")
        nc.vector.memset(pwarm[:], 0.0)
    for w in range(6):
        ptwarm = ptpool.tile([P, 2 * TQ], BF16, tag="pt", name="ptwarm")
        nc.gpsimd.memset(ptwarm[:], 0.0)

    def xt(k, c0, n):      # xT_sb [c-tile k][:, c0:c0+n] (token cols)
        return xT_sb[:, k * T + c0: k * T + c0 + n]

    def wq(k, mt):         # [128, 128] q-weight tile
        return wqkv_sb[:, k * WCOLS + mt * P: k * WCOLS + (mt + 1) * P]

    def wk(k):
        return wqkv_sb[:, k * WCOLS + QROWS: k * WCOLS + QROWS + P]

    def wv(k):
        return wqkv_sb[:, k * WCOLS + QROWS + P: k * WCOLS + QROWS + 2 * P]

    # ---- work units ----
    # During the startup window (n == 0) the attention PSUM pools are idle,
    # so first-slice projection units borrow their banks for extra overlap.
    def _ppool(pool_sel):
        if pool_sel == 1:
            return pps, "ps"
        if pool_sel == 2:
            return ppo, "po"
        return ppa, "pa"

    def unit_q(n, mt, pool_sel=0):
        def go():
            pool, tg = _ppool(pool_sel)
            ps = pool.tile([P, TQ], F32, tag=tg, name="psq")
            for k in range(KC):
                nc.tensor.matmul(ps[:], wq(k, mt), xt(k, n * TQ, TQ),
                                 start=(k == 0), stop=(k == KC - 1))
            nc.vector.tensor_scalar(
                out=q_sb[:, mt * T + n * TQ: mt * T + (n + 1) * TQ],
                in0=ps[:], scalar1=bq_sb[:, mt:mt + 1], scalar2=None, op0=add)
        return go

    def unit_k(n, pool_sel=0):
        def go():
            pool, tg = _ppool(pool_sel)
            ps = pool.tile([P, TQ], F32, tag=tg, name="psk")
            for k in range(KC):
                nc.tensor.matmul(ps[:], wk(k), xt(k, n * TQ, TQ),
                                 start=(k == 0), stop=(k == KC - 1))
            nc.vector.tensor_scalar(
                out=kT_sb[:, n * TQ:(n + 1) * TQ],
                in0=ps[:], scalar1=0.125, scalar2=bk_sb[:, 0:1],
                op0=mult, op1=add)
        return go

    def unit_v(i, pool_sel=0):
        # v_sb tile i: [0:64]=kv0, 64=ones, [65:129]=kv1, 129=ones
        def go():
            pool, tg = _ppool(pool_sel)
            ps = pool.tile([P, TQ], F32, tag=tg, name="psv")
            for k in range(KC):
                nc.tensor.matmul(ps[:, 0:P], xt(k, i * P, P), wv(k),
                                 start=(k == 0), stop=(k == KC - 1))
            nc.vector.tensor_copy(v_sb[:, i * 130: i * 130 + 64], ps[:, 0:64])
            nc.vector.tensor_copy(v_sb[:, i * 130 + 65: i * 130 + 129],
                                  ps[:, 64:128])
        return go

    def unit_attn(j, hp):
        # processes the head pair (hp, hp+4): same q/y column tile `hp`,
        # head A on partitions 0:64 (kv0), head B on 64:128 (kv1).  Their
        # score matmuls are emitted adjacently so the PE runs them
        # concurrently on disjoint row-groups (tile_position 0 vs 64).
        def go():
            nb = 4 * (j + 1)   # tk tiles in play (block-causal)
            mt = hp
            qcol = mt * T + j * TQ
            po = {}
            po[0] = ppo.tile([65, TQ], F32, tag="po", name="poA")
            po[1] = ppo.tile([65, TQ], F32, tag="po", name="poB")
            for i in range(nb):
                # ps cols [0:512] = head hp (array rows 0:64),
                #         [512:1024] = head hp+4 (array rows 64:128)
                ps = pps.tile([P, 2 * TQ], F32, tag="ps", name="pss")
                # diagonal blocks: cols < 128r are fully masked, skip them
                c0 = max(0, (i - 4 * j)) * P
                for h in (0, 1):
                    rb = 64 * h
                    nc.tensor.matmul(
                        ps[:, h * TQ + c0:(h + 1) * TQ],
                        kT_sb[rb:rb + 64, i * P:(i + 1) * P],
                        q_sb[rb:rb + 64, qcol + c0: qcol + TQ],
                        start=True, stop=True)
                pt = ptpool.tile([P, 2 * TQ], BF16, tag="pt", name="pt")
                nc.scalar.activation(pt[:, c0:2 * TQ], ps[:, c0:2 * TQ], ExpF)
                r = i - 4 * j
                if r >= 0:  # diagonal block: mask both head halves at once
                    nc.vector.tensor_tensor(
                        out=pt[:, c0:2 * TQ], in0=pt[:, c0:2 * TQ],
                        in1=maskv[r][:, c0:2 * TQ], op=mult)
                for h in (0, 1):
                    nc.tensor.matmul(
                        po[h][:, c0:TQ],
                        v_sb[:, i * 130 + 65 * h: i * 130 + 65 * h + 65],
                        pt[:, h * TQ + c0:(h + 1) * TQ],
                        start=(i == 0), stop=(i == nb - 1))
            # normalize: y = po[0:64] * broadcast(1/po[64]); copy PSUM out
            # first so the bank frees for the next head pair.
            for h in (0, 1):
                rb = 64 * h
                ob = obpool.tile([65, TQ], F32, tag="ob", name="ob")
                nc.vector.tensor_copy(ob[:], po[h][:])
                rc = rcpool.tile([1, TQ], F32, tag="rc", name="rc")
                nc.vector.reciprocal(rc[:], ob[64:65, :])
                bc = bcpool.tile([64, TQ], F32, tag="bc", name="bc")
                nc.gpsimd.partition_broadcast(bc[:], rc[:])
                nc.vector.tensor_tensor(
                    out=y_sb[rb:rb + 64, qcol: qcol + TQ],
                    in0=ob[0:64, :], in1=bc[:], op=mult)
        return go

    def unit_cproj(j, ms, ns=range(NJ)):
        def go():
            for n in ns:
                pc = ppa.tile([P, TQ], F32, tag="pa", name="pc")
                for k in range(4):
                    nc.tensor.matmul(
                        pc[:],
                        y_sb[:, k * T + j * TQ + ms * P: k * T + j * TQ + (ms + 1) * P],
                        wp_sb[:, k * C + n * TQ: k * C + (n + 1) * TQ],
                        start=(k == 0), stop=(k == 3))
                os_t = outpool.tile([P, TQ], F32, tag="os", name="os")
                nc.vector.tensor_copy(os_t[:], pc[:])
                nc.sync.dma_start(
                    out_d.ap()[j * TQ + ms * P: j * TQ + (ms + 1) * P,
                               n * TQ:(n + 1) * TQ],
                    os_t[:])
        return go

    def proj_units(n):
        return ([unit_q(n, mt) for mt in range(4)] + [unit_k(n)]
                + [unit_v(i) for i in range(4 * n, 4 * n + 4)])

    def interleave(a, b):
        """Merge unit lists evenly (a paced across b)."""
        out = []
        la, lb = len(a), len(b)
        if la == 0:
            return list(b)
        if lb == 0:
            return list(a)
        ia = ib = 0
        tot = la + lb
        for s in range(tot):
            if ia * lb <= ib * la and ia < la:
                out.append(a[ia]); ia += 1
            elif ib < lb:
                out.append(b[ib]); ib += 1
            else:
                out.append(a[ia]); ia += 1
        return out

    # ---- software-pipelined emission ----
    # P(0) first (spread over all psum pools); then per j: A(j) interleaved
    # with P(j+1) and C(j-1).
    p0 = ([unit_k(0, pool_sel=0)]
          + [unit_q(0, mt, pool_sel=[0, 1, 1, 2][mt]) for mt in range(4)]
          + [unit_v(i, pool_sel=[1, 2, 1, 0][i]) for i in range(4)])
    for u in p0:
        u()
    for j in range(NJ):
        attn = [unit_attn(j, hp) for hp in range(4)]
        filler = []
        if j + 1 < NJ:
            filler += proj_units(j + 1)
        # c_proj work is deferred one extra window where possible so the
        # ACT-bound final windows get more PE filler
        if j == NJ - 1:
            filler += [unit_cproj(jj, ms) for jj in (j - 2, j - 1)
                       for ms in range(4)]
        elif j - 1 >= 1:
            filler += [unit_cproj(j - 2, ms) for ms in range(4)]
        # keep a few filler units after the last attention unit of the
        # window so the PE has work while the final exps drain
        ntail = min(6, len(filler))
        head_f, tail_f = filler[:len(filler) - ntail], filler[len(filler) - ntail:]
        for u in interleave(attn, head_f) + tail_f:
            u()
    for ms in range(4):
        unit_cproj(NJ - 1, ms)()
    # c_proj(0) ran in window 2 via the deferred schedule; nothing left here



def _prep_inputs(x, w_attn, b_attn, w_proj):
    """Host-side shard + transpose + cast for each of the 8 cores."""
    in_maps = []
    for g in range(N_CORES):
        b, grp = divmod(g, 4)
        xT = np.ascontiguousarray(np.asarray(x[b], np.float32).T).astype(NPBF16)

        q_rows = []
        for lh in Q_ORDER:
            gh = HL * grp + lh
            q_rows.extend(range(HS * gh, HS * gh + HS))
        k0 = NE + KROWS * grp
        v0 = NE + N_KV * HS + KROWS * grp
        rows = q_rows + list(range(k0, k0 + KROWS)) + list(range(v0, v0 + KROWS))
        wqkvT = np.ascontiguousarray(w_attn[rows, :].T).astype(NPBF16)

        cols = []
        for lh in Q_ORDER:
            gh = HL * grp + lh
            cols.extend(range(HS * gh, HS * gh + HS))
        wpT = np.ascontiguousarray(w_proj[:, cols].T).astype(NPBF16)

        bq = np.asarray(b_attn[q_rows], np.float32).reshape(4, P)
        bk = (np.asarray(b_attn[k0:k0 + KROWS], np.float32) / 8.0).reshape(1, P)

        in_maps.append({"xT": xT, "wqkvT": wqkvT, "wpT": wpT,
                        "bq": bq, "bk": bk})
    return in_maps


def get_nc():
    if "nc" not in _CACHE:
        _CACHE["nc"] = _build_program()
    return _CACHE["nc"]


def kernel(x, w_attn, b_attn, w_proj, b_proj):
    x = np.asarray(x, np.float32)
    w_attn = np.asarray(w_attn, np.float32)
    b_attn = np.asarray(b_attn, np.float32)
    w_proj = np.asarray(w_proj, np.float32)
    b_proj = np.asarray(b_proj, np.float32)

    nc = get_nc()
    in_maps = _prep_inputs(x, w_attn, b_attn, w_proj)
    res = run_bass_kernel_spmd(nc, in_maps, core_ids=list(range(N_CORES)))

    # host "all-reduce" over the 4 head-group cores per batch + bias folds
    bv = b_attn[NE + N_KV * HS:]                      # [512] v bias
    bv_full = np.repeat(bv.reshape(N_KV, HS), N_HEAD // N_KV, axis=0).reshape(-1)
    delta = bv_full @ w_proj.T + b_proj               # [2048]
    out = np.zeros((B, T, C), np.float32)
    for g in range(N_CORES):
        b = g // 4
        out[b] += res.results[g]["out"]
    out += delta[None, None, :]
    return out
